# revision 22
# baseline (speedup 1.0000x reference)
"""Trainium2 Bass kernel for log-softmax multi-head attention (8 NeuronCores).

Reference computation (per batch):
    qkv = x @ w_qkv ; q,k,v per head
    dots = scale * q @ k^T ; attn = log_softmax(dots)
    out = attn @ v  -> merge heads -> out @ w_out + b_out + x

Algebraic identities used:
  1) log_softmax is linear in scores minus a row constant:
       attn = scale*dots - lse,  lse_i = ln sum_j exp(scale*dots_ij)
     so  out_head = scale * q @ (k^T v) - lse (x) colsum(v)
  2) k^T v = Wk^T (x^T x) Wv  (Gram matrix G = x^T x shared by all heads)
  3) colsum(v) = colsum(x) @ Wv
  4) the lse rank-1 correction commutes with the output projection:
       Y -= lnS_h (x) (vsum_h @ Wo_h)   summed over heads
  so the only O(n^2) work is the score matmul + exp/row-sum pass.

Sharding: 8 cores = 2 batches x 4 query-quarters. Every core computes k^T /
G for its full batch and q / lse / output for its own 1024 query rows ->
outputs disjoint, no collectives.

Schedule (the exp pass on ScalarE is the ~290us critical path):
 - dots tiles are [128, 1536] f32 double-buffered -> 6 PSUM banks, leaving
   2 banks of scratch so ALL other matmul work (kT = Wk^T x^T directly, the
   Gram chain G -> GWv -> kv, OT precompute, vsum/W8) streams underneath the
   exp window, paced by a PE-time budget.
 - exp stream is chunk-major (keys 0:1536 for all 64 (head,tile) pairs, then
   1536:3072, then 3072:4096) so the first exp only needs 3 kT chunks.
 - Ln / projections happen in a short tail (Exp and Ln live in different
   ScalarE table sets; mixing them mid-stream would thrash table loads).
"""

import numpy as np

B, N, D = 2, 4096, 512
H, DH = 8, 64
SCALE = DH**-0.5
NQ = N // 4        # own query rows per core
QT = NQ // 128     # 8 own row tiles
NXT = N // 128     # 32 x row tiles
CHUNKS = [(k, 1024) for k in range(0, 4096, 1024)]  # key ranges per exp chunk

_GRAPH_CACHE = {}


def _build_graph():
    import concourse.bass as bass
    import concourse.tile as tile
    from concourse import bacc, mybir
    from concourse.masks import make_identity

    f32 = mybir.dt.float32
    bf16 = mybir.dt.bfloat16
    AF = mybir.ActivationFunctionType

    nc = bacc.Bacc("TRN2", target_bir_lowering=False, debug=False)

    xbf_d = nc.dram_tensor("x_bf", [N, D], bf16, kind="ExternalInput").ap()
    xqbf_d = nc.dram_tensor("xq_bf", [NQ, D], bf16, kind="ExternalInput").ap()
    xq_d = nc.dram_tensor("xq", [NQ, D], f32, kind="ExternalInput").ap()
    wqkv_d = nc.dram_tensor("w_qkv_bf", [D, 3 * D], bf16, kind="ExternalInput").ap()
    wqkvf_d = nc.dram_tensor("w_qkv_f32", [D, 3 * D], f32, kind="ExternalInput").ap()
    wout_d = nc.dram_tensor("w_out_bf", [D, D], bf16, kind="ExternalInput").ap()
    bout_d = nc.dram_tensor("b_out", [D], f32, kind="ExternalInput").ap()
    out_d = nc.dram_tensor("out", [NQ, D], f32, kind="ExternalOutput").ap()

    with tile.TileContext(nc) as tc:
        with (
            tc.tile_pool(name="const", bufs=1) as const,
            tc.tile_pool(name="bigsb", bufs=1) as bigsb,
            tc.tile_pool(name="dout", bufs=2) as dout,
        ):
            # ---------------- constants + weight DMAs (issued first) -------
            ident_bf = const.tile([128, 128], bf16, tag="ident_bf")
            make_identity(nc, ident_bf[:])
            b_bc = const.tile([128, D], f32, tag="b_bc")
            nc.sync.dma_start(
                out=b_bc[:],
                in_=bass.AP(
                    tensor=bout_d.tensor,
                    offset=bout_d.offset,
                    ap=[[0, 128]] + [list(p) for p in bout_d.ap],
                ),
            )
            wq = []
            for j in range(4):
                w_t = const.tile([128, 3 * D], bf16, tag=f"wq{j}")
                nc.sync.dma_start(out=w_t[:], in_=wqkv_d[j * 128 : (j + 1) * 128, :])
                wq.append(w_t)
            wo = []
            for j in range(4):
                w_t = const.tile([128, D], bf16, tag=f"wo{j}")
                nc.sync.dma_start(out=w_t[:], in_=wout_d[j * 128 : (j + 1) * 128, :])
                wo.append(w_t)

            # own-query transposes first (they gate the very first matmuls)
            xTq = [bigsb.tile([128, NQ], bf16, name=f"xTq{j}", tag=f"xTq{j}") for j in range(4)]
            for r in range(2):
                for j in range(4):
                    nc.sync.dma_start(
                        out=xTq[j][:, r * 512 : (r + 1) * 512],
                        in_=xqbf_d[r * 512 : (r + 1) * 512, j * 128 : (j + 1) * 128],
                        transpose=True,
                    )
            # x^T via DMA xbar transposes, in key order (kT streams off these)
            xT = [bigsb.tile([128, N], bf16, name=f"xT{j}", tag=f"xT{j}") for j in range(4)]
            for r in range(8):
                for j in range(4):
                    nc.sync.dma_start(
                        out=xT[j][:, r * 512 : (r + 1) * 512],
                        in_=xbf_d[r * 512 : (r + 1) * 512, j * 128 : (j + 1) * 128],
                        transpose=True,
                    )
            # x row tiles (for the Gram matrix)
            xrow = []
            for t in range(NXT):
                xr_t = bigsb.tile([128, D], bf16, tag=f"xrow{t}")
                nc.sync.dma_start(out=xr_t[:], in_=xbf_d[t * 128 : (t + 1) * 128, :])
                xrow.append(xr_t)
            # f32 k/v weight columns (needed ~100us in, for the fp32 Gram chain)
            wf = []
            for j in range(4):
                w_t = const.tile([128, 2 * D], f32, tag=f"wf{j}")
                nc.sync.dma_start(
                    out=w_t[:], in_=wqkvf_d[j * 128 : (j + 1) * 128, 512:1536]
                )
                wf.append(w_t)
            # residual rows (f32) -- become x + b via DVE adds under the stream
            xb = []
            for t in range(QT):
                xb_t = dout.tile([128, D], f32, tag=f"xb{t}", bufs=1)
                nc.sync.dma_start(out=xb_t[:], in_=xq_d[t * 128 : (t + 1) * 128, :])
                xb.append(xb_t)

            # ---------------- big SBUF operands ----------------------------
            qT = [bigsb.tile([128, NQ], bf16, name=f"qT{c}", tag=f"qT{c}") for c in range(4)]
            kT = [bigsb.tile([128, N], bf16, name=f"kT{c}", tag=f"kT{c}") for c in range(4)]
            OT = [bigsb.tile([128, NQ], bf16, name=f"OT{c}", tag=f"OT{c}") for c in range(4)]
            G_sb = [const.tile([128, D], f32, name=f"G{j}", tag=f"G{j}") for j in range(4)]
            GWv = [const.tile([128, D], f32, name=f"GWv{j}", tag=f"GWv{j}") for j in range(4)]
            kv_p = const.tile([128, D], bf16, tag="kv_p")
            nc.vector.memset(kv_p[:], 0.0)
            csx = [const.tile([128, 1], f32, name=f"csx{j}", tag=f"csx{j}") for j in range(4)]
            csx_bf = [const.tile([128, 1], bf16, name=f"csxb{j}", tag=f"csxb{j}") for j in range(4)]
            vsT = [const.tile([128, 1], bf16, name=f"vsT{j}", tag=f"vsT{j}") for j in range(4)]
            VSmat = [const.tile([128, 8], bf16, name=f"VSm{j}", tag=f"VSm{j}") for j in range(4)]
            for j in range(4):
                nc.vector.memset(VSmat[j][:], 0.0)
            W8_sb = const.tile([8, D], bf16, tag="W8")
            lse_acc = const.tile([128, 256], f32, tag="lse_acc")
            lse_sum = const.tile([128, 64], f32, tag="lse_sum")
            lse_ln = const.tile([128, 64], bf16, tag="lse_ln")
            lnST = const.tile([8, NQ], bf16, tag="lnST")
            dummy = const.tile([128, 1], f32, tag="dummy")
            nc.vector.memset(dummy[:], 0.0)

            with (
                tc.tile_pool(name="dots_ps", bufs=1, space="PSUM") as dps,
                tc.tile_pool(name="sc_ps", bufs=1, space="PSUM") as sps,
            ):
                # preload the Exp table set before the stream
                nc.scalar.activation(out=dummy[:], in_=dummy[:], func=AF.Exp)

                def qT_half(c, nn):
                    ps = sps.tile([128, 512], f32, name="sc", tag="sc", bufs=2)
                    for j in range(4):
                        nc.tensor.matmul(
                            ps[:],
                            lhsT=wq[j][:, c * 128 : (c + 1) * 128],
                            rhs=xTq[j][:, nn * 512 : (nn + 1) * 512],
                            start=(j == 0),
                            stop=(j == 3),
                        )
                    nc.vector.tensor_copy(qT[c][:, nn * 512 : (nn + 1) * 512], ps[:])

                def kT_chunk(c, ch):
                    ps = sps.tile([128, 512], f32, name="sc", tag="sc", bufs=2)
                    for j in range(4):
                        nc.tensor.matmul(
                            ps[:],
                            lhsT=wq[j][:, 512 + c * 128 : 512 + (c + 1) * 128],
                            rhs=xT[j][:, ch * 512 : (ch + 1) * 512],
                            start=(j == 0),
                            stop=(j == 3),
                        )
                    nc.vector.tensor_copy(kT[c][:, ch * 512 : (ch + 1) * 512], ps[:])

                # ---- head: qT[0] + first 3 kT[0] chunks ---------------------
                qT_half(0, 0)
                qT_half(0, 1)
                for ch in range(2):
                    kT_chunk(0, ch)

                # ---- other-work units streamed under the exp window --------
                # (pe_cost_ns, deadline_slot, closure); deadline = stream
                # slot by which the unit MUST have been emitted (kT/qT feed
                # the dots matmuls); 999 = only needed by the tail.
                ow = []

                def add(cost, fn, dl=999):
                    ow.append((cost, dl, fn))

                for c in range(1, 4):
                    for nn in range(2):
                        add(1100, (lambda c=c, nn=nn: qT_half(c, nn)), dl=c * 16)
                # kT chunks needed at stream slot (chunk*64 + c*16)/pair rate;
                # emit low chunks of every c first.
                for chlo, chhi in ((0, 3), (3, 6), (6, 8)):
                    for c in range(4):
                        for ch in range(chlo, chhi):
                            if c == 0 and ch < 2:
                                continue
                            add(
                                1300,
                                (lambda c=c, ch=ch: kT_chunk(c, ch)),
                                dl=(ch // 2) * 64 + c * 16,
                            )
                # Gram matrix, 2 row-blocks at a time (2 scratch banks)
                for jma in (0, 2):
                    gtiles = {}
                    def g_open(gt=gtiles):
                        gt[0] = sps.tile([128, 512], f32, name="sc", tag="sc", bufs=2)
                        gt[1] = sps.tile([128, 512], f32, name="sc", tag="sc", bufs=2)
                    def g_unit(ts, jma=jma, gt=gtiles):
                        for t in ts:
                            for d in range(2):
                                jm = jma + d
                                nc.tensor.matmul(
                                    gt[d][:],
                                    lhsT=xrow[t][:, jm * 128 : (jm + 1) * 128],
                                    rhs=xrow[t][:],
                                    start=(t == 0),
                                    stop=(t == NXT - 1),
                                )
                    def g_close(jma=jma, gt=gtiles):
                        nc.vector.tensor_copy(G_sb[jma][:], gt[0][:])
                        nc.vector.tensor_copy(G_sb[jma + 1][:], gt[1][:])
                    add(0, g_open)
                    for t0 in range(0, NXT, 2):
                        add(1400, (lambda t0=t0, f=g_unit: f((t0, t0 + 1))))
                    add(100, g_close)
                # fp32 chain: GWv -> kv -> kv_p ; then vsum^T -> VSmat -> W8
                for jm in range(4):
                    def gwv_u(jm=jm):
                        ps = sps.tile([128, 512], f32, name="sc", tag="sc", bufs=2)
                        for j in range(4):
                            nc.tensor.matmul(
                                ps[:],
                                lhsT=G_sb[j][:, jm * 128 : (jm + 1) * 128],
                                rhs=wf[j][:, 512:1024],
                                start=(j == 0),
                                stop=(j == 3),
                            )
                        nc.vector.tensor_copy(GWv[jm][:], ps[:])
                    add(3600, gwv_u)
                for ha in (0, 4):
                    def kv_u(ha=ha):
                        for h in range(ha, ha + 4):
                            r0 = (h % 2) * 64
                            ps = sps.tile([128, 512], f32, name="sc", tag="sc", bufs=2)
                            for j in range(4):
                                nc.tensor.matmul(
                                    ps[0:64, 0:64],
                                    lhsT=wf[j][:, h * 64 : (h + 1) * 64],
                                    rhs=GWv[j][:, h * 64 : (h + 1) * 64],
                                    start=(j == 0),
                                    stop=(j == 3),
                                )
                            nc.vector.tensor_scalar_mul(
                                kv_p[r0 : r0 + 64, h * 64 : (h + 1) * 64],
                                ps[0:64, 0:64],
                                SCALE,
                            )
                    add(2000, kv_u)
                csx4 = [
                    const.tile([128, 4], f32, name=f"csx4_{j}", tag=f"csx4_{j}")
                    for j in range(4)
                ]
                for j in range(4):
                    for p in range(4):
                        def csx_piece(j=j, p=p):
                            nc.vector.tensor_reduce(
                                csx4[j][:, p : p + 1],
                                xT[j][:, p * 1024 : (p + 1) * 1024],
                                axis=mybir.AxisListType.X,
                                op=mybir.AluOpType.add,
                            )
                        add(100, csx_piece)
                for j in range(4):
                    def csx_fin(j=j):
                        nc.vector.tensor_reduce(
                            csx[j][:], csx4[j][:],
                            axis=mybir.AxisListType.X,
                            op=mybir.AluOpType.add,
                        )
                        nc.vector.tensor_copy(csx_bf[j][:], csx[j][:])
                    add(100, csx_fin)

                def vsum_u():
                    for jm in range(4):
                        ps = sps.tile([128, 512], f32, name="sc", tag="sc", bufs=2)
                        for j in range(4):
                            nc.tensor.matmul(
                                ps[:, 0:1],
                                lhsT=wq[j][:, 1024 + jm * 128 : 1024 + (jm + 1) * 128],
                                rhs=csx_bf[j][:],
                                start=(j == 0),
                                stop=(j == 3),
                            )
                        # negated: W8 rows become -(vsum_h @ Wo_h)
                        nc.vector.tensor_scalar_mul(vsT[jm][:], ps[:, 0:1], -1.0)
                    for j in range(4):
                        nc.vector.tensor_copy(
                            VSmat[j][0:64, 2 * j : 2 * j + 1], vsT[j][0:64, :]
                        )
                        nc.vector.tensor_copy(
                            VSmat[j][64:128, 2 * j + 1 : 2 * j + 2], vsT[j][64:128, :]
                        )
                add(1500, vsum_u)
                def w8_u():
                    ps = sps.tile([128, 512], f32, name="sc", tag="sc", bufs=2)
                    for j in range(4):
                        nc.tensor.matmul(
                            ps[0:8, :],
                            lhsT=VSmat[j][:],
                            rhs=wo[j][:],
                            start=(j == 0),
                            stop=(j == 3),
                        )
                    nc.vector.tensor_copy(W8_sb[:], ps[0:8, :])
                add(1300, w8_u)
                # OT precompute (kv part of the output, transposed layout)
                for h in range(H):
                    def ot_u(h=h):
                        c, r0 = h // 2, (h % 2) * 64
                        for nn in range(2):
                            ps = sps.tile([128, 512], f32, name="sc", tag="sc", bufs=2)
                            nc.tensor.matmul(
                                ps[r0 : r0 + 64, :],
                                lhsT=kv_p[:, h * 64 : (h + 1) * 64],
                                rhs=qT[c][:, nn * 512 : (nn + 1) * 512],
                                start=True,
                                stop=True,
                            )
                            nc.vector.tensor_copy(
                                OT[c][r0 : r0 + 64, nn * 512 : (nn + 1) * 512],
                                ps[r0 : r0 + 64, :],
                            )
                    add(1000, ot_u)
                # x + b residual prep (DVE only)
                for t in range(QT):
                    add(0, (lambda t=t: nc.vector.tensor_add(
                        xb[t][:], xb[t][:], b_bc[:])))

                # ---- the exp stream ---------------------------------------
                # pacing: honour hard deadlines, otherwise spread the units
                # evenly so they drain by ~slot 150 of 192.
                rate = len(ow) / 200.0
                popped = [0]

                def pump(slot):
                    while ow:
                        overdue = min(d for _, d, _ in ow) <= slot
                        ahead = popped[0] < (slot + 1) * rate
                        if not (overdue or ahead):
                            break
                        _, _, fn = ow.pop(0)
                        fn()
                        popped[0] += 1

                for ci, (k0, fd) in enumerate(CHUNKS):
                    for c in range(4):
                        for t in range(QT):
                            for hp in range(2):
                                h, r0 = 2 * c + hp, hp * 64
                                slot = ci * 64 + c * 16 + t * 2 + hp
                                pump(slot)
                                dtile = dps.tile([128, 1024], f32, name="dots", tag="dots", bufs=3)
                                lhsT = qT[c][r0 : r0 + 64, t * 128 : (t + 1) * 128]
                                for s0 in range(0, fd, 512):
                                    nc.tensor.matmul(
                                        dtile[:, s0 : s0 + 512],
                                        lhsT=lhsT,
                                        rhs=kT[c][r0 : r0 + 64, k0 + s0 : k0 + s0 + 512],
                                        start=True,
                                        stop=True,
                                    )
                                col = ci * 64 + t * 8 + h
                                nc.scalar.activation(
                                    out=dtile[:, 0:fd],
                                    in_=dtile[:, 0:fd],
                                    func=AF.Exp,
                                    scale=SCALE,
                                    accum_out=lse_acc[:, col : col + 1],
                                )

                # force-drain any remaining units
                for _, _, fn in ow:
                    fn()
                ow = []

                # ---- tail: lse -> Ln -> -lnS^T ; projection + residual -----
                la = lse_acc[:].rearrange("q (four p) -> q four p", four=4)
                nc.vector.tensor_add(lse_sum[:], la[:, 0, :], la[:, 1, :])
                nc.vector.tensor_add(lse_sum[:], lse_sum[:], la[:, 2, :])
                nc.vector.tensor_add(lse_sum[:], lse_sum[:], la[:, 3, :])
                nc.scalar.activation(out=lse_ln[:], in_=lse_sum[:], func=AF.Ln)
                for t in range(QT):
                    ps = sps.tile([128, 512], f32, name="sc", tag="sc", bufs=2)
                    ps_bf = ps[0:8, 0:64].bitcast(bf16)
                    nc.tensor.transpose(ps_bf, lse_ln[:, t * 8 : (t + 1) * 8], ident_bf[:])
                    nc.vector.tensor_copy(lnST[:, t * 128 : (t + 1) * 128], ps_bf)
                for t in range(QT):
                    yps = sps.tile([128, 512], f32, name="sc", tag="sc", bufs=2)
                    for c in range(4):
                        nc.tensor.matmul(
                            yps[:],
                            lhsT=OT[c][:, t * 128 : (t + 1) * 128],
                            rhs=wo[c][:],
                            start=(c == 0),
                            stop=False,
                        )
                    nc.tensor.matmul(
                        yps[:],
                        lhsT=lnST[:, t * 128 : (t + 1) * 128],
                        rhs=W8_sb[:],
                        start=False,
                        stop=True,
                    )
                    ysb = dout.tile([128, D], f32, name="ysb", tag="ysb")
                    nc.vector.tensor_add(ysb[:], yps[:], xb[t][:])
                    nc.sync.dma_start(out=out_d[t * 128 : (t + 1) * 128, :], in_=ysb[:])

    nc.compile()
    return nc


def get_graph():
    if "nc" not in _GRAPH_CACHE:
        _GRAPH_CACHE["nc"] = _build_graph()
    return _GRAPH_CACHE["nc"]


def make_in_maps(x, w_qkv, w_out, b_out):
    import ml_dtypes

    x = np.ascontiguousarray(x, dtype=np.float32)
    w_qkv = np.ascontiguousarray(w_qkv, dtype=np.float32)
    w_out = np.ascontiguousarray(w_out, dtype=np.float32)
    b_out = np.ascontiguousarray(b_out, dtype=np.float32)
    x_bf = x.astype(ml_dtypes.bfloat16)
    w_qkv_bf = w_qkv.astype(ml_dtypes.bfloat16)
    w_out_bf = w_out.astype(ml_dtypes.bfloat16)
    in_maps = []
    for i in range(8):
        b, q = divmod(i, 4)
        in_maps.append(
            {
                "x_bf": x_bf[b],
                "xq_bf": np.ascontiguousarray(x_bf[b, q * NQ : (q + 1) * NQ]),
                "xq": np.ascontiguousarray(x[b, q * NQ : (q + 1) * NQ]),
                "w_qkv_bf": w_qkv_bf,
                "w_qkv_f32": w_qkv,
                "w_out_bf": w_out_bf,
                "b_out": b_out,
            }
        )
    return in_maps


def kernel(x, w_qkv, w_out, b_out):
    from concourse.bass_utils import run_bass_kernel_spmd

    nc = get_graph()
    in_maps = make_in_maps(x, w_qkv, w_out, b_out)
    res = run_bass_kernel_spmd(nc, in_maps, core_ids=list(range(8)))
    out = np.empty((B, N, D), np.float32)
    for i in range(8):
        b, q = divmod(i, 4)
        out[b, q * NQ : (q + 1) * NQ] = res.results[i]["out"]
    return out
